# revision 21
# baseline (speedup 1.0000x reference)
"""Trainium2 Bass kernel for nn_CrossAttention (dual cross-attention + groupnorm).

Sharding: 8 branch-batches (2 branches x 4 batch) -> 8 cores, one full
cross-attention per core. Core c: branch = c // 4 ('a' if 0 else 'b'),
batch = c % 4.

Per-core math (x_q, x_kv are [C=256, N=4096]):
  q = (Wq x_q + bq) * SCALE, k = Wk x_kv + bk, v = Wv x_kv + bv
  sT[j, i] = sum_d k[d, j] q[d, i]  per head  (PE row-tiled, 4 heads packed)
  E = exp(sT)                        (ACT; |s| << 1 so no max-subtraction)
  u[d, i] = sum_j v[d, j] E[j, i]; colsum[i] = sum_j E[j, i]
            (PE col-tiled, 4 heads packed; colsum via ones-column in vT)
  attn = u / colsum ; out = GN(x_q + Wo attn + ob) * gamma + beta

Head h lives on partitions 32h..32h+15 for q/k. vT j-tiles are [128, 128]
with head h in cols 32h..32h+15, ones at col 32h+16, zeros elsewhere.

Hardware constraint handled throughout: a Matmult instruction may carry at
most ONE semaphore wait, and Tile does not transitively reduce waits. So:
one DMA per input tensor; tiny PE "warmup" matmuls absorb each DMA
semaphore individually; a single PSUM pool with two persistent tags (no
pool release boundaries); ACT zero-fill copies shield matmul first-writes
into recycled PSUM slots so the matmul waits only on the ACT queue.
"""

import sys

sys.path.insert(0, "/opt/trn_rl_repo")

import numpy as np
import ml_dtypes

import concourse.bass as bass
import concourse.bacc as bacc
import concourse.tile as tile
from concourse import mybir

F32 = mybir.dt.float32
BF16 = mybir.dt.bfloat16

B, C, HW, N = 4, 256, 64, 4096
PROJ, HEADS, HD = 64, 4, 16
SCALE = HD ** -0.5
GROUPS, EPS = 16, 1e-5
NCORES = 8
IPP = 4              # i-chunks per pass (PSUM: 4 score banks + 4 pv banks)


def build_nc(n=N):
    jt, ich = n // 128, n // 512
    ipp = min(IPP, ich)
    passes = ich // ipp
    gn_cnt = float((C // GROUPS) * n)  # elements per group

    nc = bacc.Bacc(None, target_bir_lowering=False)

    x_q = nc.declare_dram_parameter("x_q", [128, 2, n], F32, isOutput=False)
    x_kv = nc.declare_dram_parameter("x_kv", [128, 2, n], F32, isOutput=False)
    wq_p = nc.declare_dram_parameter("wq", [128, 256], F32, isOutput=False)
    wk_p = nc.declare_dram_parameter("wk", [128, 256], F32, isOutput=False)
    wv_p = nc.declare_dram_parameter("wv", [128, 256], F32, isOutput=False)
    wo_p = nc.declare_dram_parameter("wo", [128, 256], BF16, isOutput=False)
    bq_p = nc.declare_dram_parameter("bq", [1, 128], F32, isOutput=False)
    bk_p = nc.declare_dram_parameter("bk", [1, 128], F32, isOutput=False)
    bv_p = nc.declare_dram_parameter("bv", [1, 128], F32, isOutput=False)
    bo_p = nc.declare_dram_parameter("bo", [1, 256], F32, isOutput=False)
    g16_p = nc.declare_dram_parameter("g16", [128, 32], F32, isOutput=False)
    gb_p = nc.declare_dram_parameter("gb", [128, 4], F32, isOutput=False)
    out = nc.declare_dram_parameter("out", [2, 128, n], F32, isOutput=True)

    cs_dram = nc.dram_tensor("cs_scratch", [passes, ipp, 4, 512], F32)
    r_dram = nc.dram_tensor("r_scratch", [passes, ipp, 4, 512], F32)
    mr_dram = nc.dram_tensor("mr_scratch", [16, 2], F32)

    ADD = mybir.AluOpType.add
    MUL = mybir.AluOpType.mult
    SUB = mybir.AluOpType.subtract
    EXP = mybir.ActivationFunctionType.Exp
    SQRT = mybir.ActivationFunctionType.Sqrt
    COPY = mybir.ActivationFunctionType.Copy

    with tile.TileContext(nc) as tc:
        with tc.tile_pool(name="wpool", bufs=1) as wp, \
             tc.tile_pool(name="psum", space="PSUM", bufs=1) as pp, \
             tc.tile_pool(name="bigsb", bufs=1) as bp, \
             tc.tile_pool(name="epool", bufs=3) as ep, \
             tc.tile_pool(name="rpool", bufs=2) as rp, \
             tc.tile_pool(name="spool", bufs=1) as sp, \
             tc.tile_pool(name="opool", bufs=2) as op:

            def pvtile(name):
                return pp.tile([128, 512], F32, tag="pv", bufs=4, name=name,
                               uniquify=True)

            def zfill(t):
                pt, ft = t.shape[0], t.shape[-1]
                nc.scalar.activation(t, zeros_sb[:pt, :ft], COPY)

            wq_sb = wp.tile([128, 256], F32)
            wk_sb = wp.tile([128, 256], F32)
            wv_sb = wp.tile([128, 256], F32)
            wo_sb = wp.tile([128, 256], BF16)
            g16_sb = wp.tile([128, 32], F32)
            gb_sb = wp.tile([128, 4], F32)
            bq_sb = wp.tile([1, 128], F32)
            bk_sb = wp.tile([1, 128], F32)
            bv_sb = wp.tile([1, 128], F32)
            bo_sb = wp.tile([1, 256], F32)
            ones_n = wp.tile([1, 512], F32)
            zeros_sb = wp.tile([128, 512], F32)
            fence_sb = wp.tile([1, 1], F32)
            nc.vector.memset(ones_n, 1.0)
            nc.vector.memset(zeros_sb, 0.0)
            nc.vector.memset(fence_sb, 0.0)
            nc.sync.dma_start(out=wq_sb, in_=wq_p[:])
            nc.sync.dma_start(out=wk_sb, in_=wk_p[:])
            nc.sync.dma_start(out=wv_sb, in_=wv_p[:])
            nc.sync.dma_start(out=wo_sb, in_=wo_p[:])
            nc.sync.dma_start(out=g16_sb, in_=g16_p[:])
            nc.sync.dma_start(out=gb_sb, in_=gb_p[:])
            nc.sync.dma_start(out=bq_sb, in_=bq_p[:])
            nc.sync.dma_start(out=bk_sb, in_=bk_p[:])
            nc.sync.dma_start(out=bv_sb, in_=bv_p[:])
            nc.sync.dma_start(out=bo_sb, in_=bo_p[:])

            xq_sb = bp.tile([128, 2, n], F32)
            xkv_sb = bp.tile([128, 2, n], F32)
            q_sb = bp.tile([128, n], BF16)
            k_sb = bp.tile([128, n], BF16)
            vt_sb = bp.tile([128, n], BF16)
            attn_sb = bp.tile([128, n], BF16)
            y_sb = bp.tile([128, 2, n], F32)

            nc.sync.dma_start(out=xq_sb, in_=x_q[:])
            nc.sync.dma_start(out=xkv_sb, in_=x_kv[:])

            # PE warmups: absorb each input-DMA semaphore on its own matmul
            # (distinct columns of one PSUM tile -> no WAW between them).
            warm = pvtile("warm")
            warm_srcs = (xq_sb[:, 0, 0:1], xkv_sb[:, 0, 0:1],
                         wq_sb[:, 0:1], wk_sb[:, 0:1], wv_sb[:, 0:1],
                         g16_sb[:, 0:1], wo_sb[:, 0:1], bo_sb[0:1, 0:1],
                         zeros_sb[:, 0:1])
            for wi, wt in enumerate(warm_srcs):
                nc.tensor.matmul(warm[0:1, wi:wi + 1], wt, wt,
                                 start=True, stop=True, skip_group_check=True)

            # ---------- stage A: projections (vT first, so later q/k DVE
            # evacuation ticks cover the vT ticks for the main loop) ----------
            for j in range(jt):
                js = slice(128 * j, 128 * j + 128)
                pv = pvtile("pv")
                for cc in range(2):
                    nc.tensor.matmul(
                        pv[:, 0:128], xkv_sb[:, cc, js],
                        wv_sb[:, 128 * cc:128 * cc + 128],
                        start=(cc == 0), stop=False)
                nc.tensor.matmul(pv[:, 0:128], ones_n[0:1, 0:128], bv_sb,
                                 start=False, stop=True)
                nc.vector.tensor_copy(vt_sb[:, js], pv[:, 0:128])

            for nchunk in range(n // 512):
                s = slice(512 * nchunk, 512 * nchunk + 512)
                for (w_sb, b_sb, src, dst) in (
                    (wq_sb, bq_sb, xq_sb, q_sb),
                    (wk_sb, bk_sb, xkv_sb, k_sb),
                ):
                    ps = pvtile("ps")
                    for cc in range(2):
                        nc.tensor.matmul(
                            ps, w_sb[:, 128 * cc:128 * cc + 128],
                            src[:, cc, s], start=(cc == 0), stop=False)
                    nc.tensor.matmul(ps, b_sb, ones_n, start=False, stop=True)
                    nc.vector.tensor_copy(dst[:, s], ps)

            # DVE fence + absorber: the first (mode-switching) QK matmul must
            # carry a PE wait, so absorb the q/k-evacuation DVE tick here.
            nc.vector.tensor_copy(fence_sb, k_sb[0:1, n - 1:n])
            nc.tensor.matmul(warm[0:1, 9:10], fence_sb, fence_sb,
                             start=True, stop=True, skip_group_check=True)

            # ---------- main loop: QK -> exp -> PV ----------
            for p_i in range(passes):
                pvs = [pvtile(f"pvacc{p_i}_{i}") for i in range(ipp)]
                # ACT zero-fill: absorbs this PSUM slot's previous readers/
                # writers on the ACT queue; also provides the zero base the
                # start=False accumulation needs.
                for ic in range(ipp):
                    zfill(pvs[ic])
                for j in range(jt):
                    js = slice(128 * j, 128 * j + 128)
                    for ic in range(ipp):
                        i0 = 512 * (ipp * p_i + ic)
                        isl = slice(i0, i0 + 512)
                        sc = pp.tile([128, 2048], F32, tag="sc", bufs=1,
                                     name="sc")
                        for h in range(4):
                            hp = slice(32 * h, 32 * h + 16)
                            nc.tensor.matmul(
                                sc[:, 512 * h:512 * h + 512],
                                k_sb[hp, js], q_sb[hp, isl],
                                start=True, stop=True,
                                tile_position=(32 * h, 0))
                        e_t = ep.tile([128, 2048], BF16, tag="e", name="e_t")
                        nc.scalar.activation(e_t, sc, EXP)
                        for h in range(4):
                            nc.tensor.matmul(
                                pvs[ic][32 * h:32 * h + 32, :],
                                vt_sb[:, 128 * j + 32 * h:128 * j + 32 * h + 32],
                                e_t[:, 512 * h:512 * h + 512],
                                start=False, stop=(j == jt - 1),
                                tile_position=(0, 32 * h),
                                skip_group_check=True)
                # absorb the pending PE writes of each accumulator on a
                # single-wait matmul each, before any DVE reader touches them
                # (adds 0 to a padding-derived element; numerically inert).
                for ic in range(ipp):
                    nc.tensor.matmul(pvs[ic][0:1, 0:1], zeros_sb[0:1, 0:1],
                                     zeros_sb[0:1, 0:1], start=False, stop=False,
                                     skip_group_check=True)
                # pass epilogue: colsums -> reciprocal -> normalize
                for ic in range(ipp):
                    cs_sb = rp.tile([128, 512], F32, tag="cs", name="cs_sb")
                    nc.vector.tensor_copy(cs_sb, pvs[ic])
                    for h in range(4):
                        nc.sync.dma_start(
                            out=cs_dram[p_i, ic, h],
                            in_=cs_sb[32 * h + 16:32 * h + 17, :])
                csrows = ipp * 4 * 512 // 64
                cs_p = rp.tile([csrows, 64], F32, tag="csp", name="cs_p")
                nc.sync.dma_start(
                    out=cs_p,
                    in_=cs_dram[p_i].rearrange("a b (g f) -> (a b g) f", f=64))
                r_p = rp.tile([csrows, 64], F32, tag="csp", name="r_p")
                nc.vector.reciprocal(r_p, cs_p)
                nc.sync.dma_start(
                    out=r_dram[p_i].rearrange("a b (g f) -> (a b g) f", f=64),
                    in_=r_p)
                for ic in range(ipp):
                    i0 = 512 * (ipp * p_i + ic)
                    rr = rp.tile([128, 512], F32, tag="rr", name="rr")
                    nc.sync.dma_start(
                        out=rr,
                        in_=bass.AP(r_dram, (p_i * ipp + ic) * 4 * 512,
                                    [[512, 4], [0, 32], [1, 512]]))
                    nc.vector.tensor_tensor(
                        attn_sb[:, i0:i0 + 512], pvs[ic], rr, MUL)
                # DVE fence + absorber: a PE matmul whose only fresh
                # dependency is the latest DVE tick of this pass's epilogue
                # (RAW on the last attn slice orders the fence last).
                i0_last = 512 * (ipp * p_i + ipp - 1)
                nc.vector.tensor_copy(fence_sb,
                                      attn_sb[0:1, i0_last + 511:i0_last + 512])
                nc.tensor.matmul(pvs[0][0:1, 1:2], fence_sb, fence_sb,
                                 start=False, stop=False, skip_group_check=True)

            # ---------- stage C: out-proj + residual + groupnorm ----------
            for ic in range(ich):
                isl = slice(512 * ic, 512 * ic + 512)
                for ct in range(2):
                    pz = pvtile("pz")
                    nc.tensor.matmul(pz, wo_sb[:, 128 * ct:128 * ct + 128],
                                     attn_sb[:, isl], start=True, stop=False)
                    nc.tensor.matmul(pz, bo_sb[0:1, 128 * ct:128 * ct + 128],
                                     ones_n, start=False, stop=True)
                    nc.vector.tensor_tensor(
                        y_sb[:, ct, isl], pz, xq_sb[:, ct, isl], ADD)

            m1 = pvtile("m1")
            m2 = pvtile("m2")
            for ct in range(2):
                y2 = op.tile([128, n], F32, tag="y2", bufs=1, name="y2")
                nc.vector.tensor_tensor(y2, y_sb[:, ct, :], y_sb[:, ct, :], MUL)
                for ch in range(n // 512):
                    s = slice(512 * ch, 512 * ch + 512)
                    first = ct == 0 and ch == 0
                    last = ct == 1 and ch == n // 512 - 1
                    nc.tensor.matmul(m1[:16, :], g16_sb[:, 16 * ct:16 * ct + 16],
                                     y_sb[:, ct, s], start=first, stop=last)
                    nc.tensor.matmul(m2[:16, :], g16_sb[:, 16 * ct:16 * ct + 16],
                                     y2[:, s], start=first, stop=last)

            mv = sp.tile([16, 2], F32, name="mv")
            nc.vector.reduce_sum(mv[:, 0:1], m1[:16, :],
                                 axis=mybir.AxisListType.X)
            nc.vector.reduce_sum(mv[:, 1:2], m2[:16, :],
                                 axis=mybir.AxisListType.X)
            mean = sp.tile([16, 1], F32, name="mean")
            e2 = sp.tile([16, 1], F32, name="e2")
            var = sp.tile([16, 1], F32, name="var")
            sd = sp.tile([16, 1], F32, name="sd")
            rstd = sp.tile([16, 1], F32, name="rstd")
            eps_t = sp.tile([16, 1], F32, name="eps_t")
            mr = sp.tile([16, 2], F32, name="mr")
            nc.vector.memset(eps_t, EPS)
            nc.vector.tensor_scalar_mul(mean, mv[:, 0:1], 1.0 / gn_cnt)
            nc.vector.tensor_scalar_mul(e2, mv[:, 1:2], 1.0 / gn_cnt)
            nc.vector.tensor_tensor(var, mean, mean, MUL)
            nc.vector.tensor_tensor(var, e2, var, SUB)
            nc.scalar.activation(sd, var, SQRT, bias=eps_t)
            nc.vector.reciprocal(rstd, sd)
            nc.vector.tensor_copy(mr[:, 0:1], mean)
            nc.vector.tensor_copy(mr[:, 1:2], rstd)
            nc.sync.dma_start(out=mr_dram[:], in_=mr)

            for ct in range(2):
                mrb = sp.tile([128, 2], F32, tag="mrb", name="mrb")
                nc.sync.dma_start(
                    out=mrb,
                    in_=bass.AP(mr_dram, 16 * ct, [[2, 8], [0, 16], [1, 2]]))
                rg = sp.tile([128, 1], F32, tag="rg", name="rg")
                bb = sp.tile([128, 1], F32, tag="bb", name="bb")
                nc.vector.tensor_tensor(rg, mrb[:, 1:2],
                                        gb_sb[:, 2 * ct:2 * ct + 1], MUL)
                nc.vector.tensor_tensor(bb, mrb[:, 0:1], rg, MUL)
                nc.vector.tensor_tensor(bb, gb_sb[:, 2 * ct + 1:2 * ct + 2],
                                        bb, SUB)
                for half in range(max(1, n // 2048)):
                    hs = slice(2048 * half, min(2048 * half + 2048, n))
                    o_t = op.tile([128, 2048], F32, tag="o", name="o_t")
                    width = hs.stop - hs.start
                    nc.vector.tensor_scalar(
                        o_t[:, :width], y_sb[:, ct, hs], rg, bb, MUL, ADD)
                    nc.sync.dma_start(out=out[ct][:, hs], in_=o_t[:, :width])
    nc.finalize()
    return nc


# ---------------- host side ----------------

def _prep_core(x_q, x_kv, wq, bq, wk, bk, wv, bv, wo, bo, gamma, beta):
    d = {}
    d["x_q"] = np.ascontiguousarray(
        x_q.reshape(2, 128, -1).transpose(1, 0, 2)).astype(np.float32)
    d["x_kv"] = np.ascontiguousarray(
        x_kv.reshape(2, 128, -1).transpose(1, 0, 2)).astype(np.float32)

    def lhsT_packed(w, scale):
        lt = np.zeros((C, 128), np.float32)
        for h in range(HEADS):
            lt[:, 32 * h:32 * h + HD] = scale * w[HD * h:HD * h + HD, :].T
        return np.ascontiguousarray(
            lt.reshape(2, 128, 128).transpose(1, 0, 2).reshape(128, 256))

    d["wq"] = lhsT_packed(wq, SCALE)
    d["wk"] = lhsT_packed(wk, 1.0)

    def brow(b, scale):
        r = np.zeros((1, 128), np.float32)
        for h in range(HEADS):
            r[0, 32 * h:32 * h + HD] = scale * b[HD * h:HD * h + HD]
        return r

    d["bq"] = brow(bq, SCALE)
    d["bk"] = brow(bk, 1.0)

    wv_aug = np.zeros((C, 128), np.float32)
    bv_aug = np.zeros((1, 128), np.float32)
    for h in range(HEADS):
        wv_aug[:, 32 * h:32 * h + HD] = wv[HD * h:HD * h + HD, :].T
        bv_aug[0, 32 * h:32 * h + HD] = bv[HD * h:HD * h + HD]
        bv_aug[0, 32 * h + HD] = 1.0
    d["wv"] = np.ascontiguousarray(
        wv_aug.reshape(2, 128, 128).transpose(1, 0, 2).reshape(128, 256))
    d["bv"] = bv_aug

    wo_pad = np.zeros((128, C), np.float32)  # [r=32h+d, c]
    for h in range(HEADS):
        wo_pad[32 * h:32 * h + HD, :] = wo[:, HD * h:HD * h + HD].T
    d["wo"] = np.ascontiguousarray(wo_pad).astype(ml_dtypes.bfloat16)
    d["bo"] = bo.reshape(1, 256).astype(np.float32)

    g16 = np.zeros((128, 32), np.float32)
    for ct in range(2):
        for r in range(128):
            g16[r, 16 * ct + 8 * ct + r // 16] = 1.0
    d["g16"] = g16
    gb = np.zeros((128, 4), np.float32)
    for ct in range(2):
        gb[:, 2 * ct] = gamma.reshape(2, 128)[ct]
        gb[:, 2 * ct + 1] = beta.reshape(2, 128)[ct]
    d["gb"] = gb
    return d


_CACHE = {}


def _get_nc(n=N):
    if n not in _CACHE:
        _CACHE[n] = build_nc(n)
    return _CACHE[n]


class _Runner:
    """run_bass_via_pjrt with the jitted executable cached across calls."""

    def __init__(self, nc, n_cores=NCORES):
        import jax
        import jax.numpy as jnp
        from jax.sharding import Mesh, PartitionSpec
        from jax.experimental.shard_map import shard_map
        from concourse import bass2jax
        from concourse import mybir as mb

        bass2jax.install_neuronx_cc_hook()
        self.nc = nc
        self.n_cores = n_cores
        partition_name = (nc.partition_id_tensor.name
                          if nc.partition_id_tensor else None)
        in_names, out_names, out_avals, zero_outs = [], [], [], []
        for alloc in nc.m.functions[0].allocations:
            if not isinstance(alloc, mb.MemoryLocationSet):
                continue
            name = alloc.memorylocations[0].name
            if alloc.kind == "ExternalInput":
                if name != partition_name:
                    in_names.append(name)
            elif alloc.kind == "ExternalOutput":
                out_names.append(name)
                shape = tuple(alloc.tensor_shape)
                dtype = mb.dt.np(alloc.dtype)
                out_avals.append(jax.core.ShapedArray(shape, dtype))
                zero_outs.append(np.zeros(shape, dtype))
        self.in_names, self.out_names = in_names, out_names
        self.zero_outs = zero_outs
        n_params, n_outs = len(in_names), len(out_names)
        donate = tuple(range(n_params, n_params + n_outs))

        def _body(*args):
            operands = list(args)
            all_in_names = list(in_names) + list(out_names)
            if partition_name is not None:
                operands.append(bass2jax.partition_id_tensor())
                all_in_names.append(partition_name)
            outs = bass2jax._bass_exec_p.bind(
                *operands,
                out_avals=tuple(out_avals),
                in_names=tuple(all_in_names),
                out_names=tuple(out_names),
                lowering_input_output_aliases=(),
                sim_require_finite=True,
                sim_require_nnan=True,
                nc=nc,
            )
            return tuple(outs)

        devices = jax.devices()[:n_cores]
        mesh = Mesh(np.asarray(devices), ("core",))
        in_specs = (PartitionSpec("core"),) * (n_params + n_outs)
        out_specs = (PartitionSpec("core"),) * n_outs
        self.fn = jax.jit(
            shard_map(_body, mesh=mesh, in_specs=in_specs,
                      out_specs=out_specs, check_rep=False),
            donate_argnums=donate, keep_unused=True)

    def __call__(self, in_maps, block=True):
        ins = [
            np.concatenate([np.asarray(m[name]) for m in in_maps], axis=0)
            for name in self.in_names
        ]
        zouts = [np.concatenate([z] * self.n_cores, axis=0)
                 for z in self.zero_outs]
        outs = self.fn(*ins, *zouts)
        if block:
            for o in outs:
                o.block_until_ready()
        per_core = []
        for c in range(self.n_cores):
            d = {}
            for name, arr, zo in zip(self.out_names, outs, self.zero_outs):
                k = zo.shape[0]
                d[name] = np.asarray(arr[c * k:(c + 1) * k])
            per_core.append(d)
        return per_core


_RUNNER = {}


def get_runner(n=N):
    if n not in _RUNNER:
        _RUNNER[n] = _Runner(_get_nc(n))
    return _RUNNER[n]


def run_cores(in_maps, n=N):
    return get_runner(n)(in_maps)


def make_in_maps(feat_a, feat_b, weights):
    w = weights
    in_maps = []
    for core in range(NCORES):
        br, b = core // 4, core % 4
        if br == 0:
            d = _prep_core(
                feat_a[b].reshape(C, -1), feat_b[b].reshape(C, -1),
                w["q_a_w"], w["q_a_b"], w["k_b_w"], w["k_b_b"],
                w["v_b_w"], w["v_b_b"], w["out_a_w"], w["out_a_b"],
                w["norm_a_g"], w["norm_a_b"])
        else:
            d = _prep_core(
                feat_b[b].reshape(C, -1), feat_a[b].reshape(C, -1),
                w["q_b_w"], w["q_b_b"], w["k_a_w"], w["k_a_b"],
                w["v_a_w"], w["v_a_b"], w["out_b_w"], w["out_b_b"],
                w["norm_b_g"], w["norm_b_b"])
        in_maps.append({k: np.ascontiguousarray(v) for k, v in d.items()})
    return in_maps


def kernel(**inputs):
    feat_a = np.asarray(inputs["feat_a"], np.float32)
    feat_b = np.asarray(inputs["feat_b"], np.float32)
    in_maps = make_in_maps(feat_a, feat_b, inputs)
    results = run_cores(in_maps)

    def unpack(r):
        return r["out"].reshape(C, HW, HW)

    a_out = np.stack([unpack(results[b]) for b in range(4)])
    b_out = np.stack([unpack(results[4 + b]) for b in range(4)])
    return (a_out, b_out)


# revision 22
# speedup vs baseline: 106.0701x; 106.0701x over previous
"""Trainium2 Bass kernel for nn_CrossAttention (dual cross-attention + groupnorm).

Sharding: 8 branch-batches (2 branches x 4 batch) -> 8 cores, one full
cross-attention per core. Core c: branch = c // 4 ('a' if 0 else 'b'),
batch = c % 4.

Per-core math (x_q, x_kv are [C=256, N=4096]):
  q = (Wq x_q + bq) * SCALE, k = Wk x_kv + bk, v = Wv x_kv + bv
  sT[j, i] = sum_d k[d, j] q[d, i]  per head  (PE row-tiled, 4 heads packed)
  E = exp(sT)                        (ACT; |s| << 1 so no max-subtraction)
  u[d, i] = sum_j v[d, j] E[j, i]; colsum[i] = sum_j E[j, i]
            (PE col-tiled, 4 heads packed; colsum via ones-column in vT)
  attn = u / colsum ; out = GN(x_q + Wo attn + ob) * gamma + beta

Head h lives on partitions 32h..32h+15 for q/k. vT j-tiles are [128, 128]
with head h in cols 32h..32h+15, ones at col 32h+16, zeros elsewhere.

Hardware constraint handled throughout: a Matmult instruction may carry at
most ONE semaphore wait, and Tile does not transitively reduce waits. So:
one DMA per input tensor; tiny PE "warmup" matmuls absorb each DMA
semaphore individually; a single PSUM pool with two persistent tags (no
pool release boundaries); ACT zero-fill copies shield matmul first-writes
into recycled PSUM slots so the matmul waits only on the ACT queue.
"""

import sys

sys.path.insert(0, "/opt/trn_rl_repo")

import numpy as np
import ml_dtypes

import concourse.bass as bass
import concourse.bacc as bacc
import concourse.tile as tile
from concourse import mybir

F32 = mybir.dt.float32
BF16 = mybir.dt.bfloat16

B, C, HW, N = 4, 256, 64, 4096
PROJ, HEADS, HD = 64, 4, 16
SCALE = HD ** -0.5
GROUPS, EPS = 16, 1e-5
NCORES = 8
IPP = 4              # i-chunks per pass (PSUM: 4 score banks + 4 pv banks)


def build_nc(n=N):
    jt, ich = n // 128, n // 512
    ipp = min(IPP, ich)
    passes = ich // ipp
    gn_cnt = float((C // GROUPS) * n)  # elements per group

    nc = bacc.Bacc(None, target_bir_lowering=False)

    x_q = nc.declare_dram_parameter("x_q", [128, 2, n], F32, isOutput=False)
    x_kv = nc.declare_dram_parameter("x_kv", [128, 2, n], F32, isOutput=False)
    wq_p = nc.declare_dram_parameter("wq", [128, 256], F32, isOutput=False)
    wk_p = nc.declare_dram_parameter("wk", [128, 256], F32, isOutput=False)
    wv_p = nc.declare_dram_parameter("wv", [128, 256], F32, isOutput=False)
    wo_p = nc.declare_dram_parameter("wo", [128, 256], BF16, isOutput=False)
    bq_p = nc.declare_dram_parameter("bq", [1, 128], F32, isOutput=False)
    bk_p = nc.declare_dram_parameter("bk", [1, 128], F32, isOutput=False)
    bv_p = nc.declare_dram_parameter("bv", [1, 128], F32, isOutput=False)
    bo_p = nc.declare_dram_parameter("bo", [1, 256], F32, isOutput=False)
    g16_p = nc.declare_dram_parameter("g16", [128, 32], F32, isOutput=False)
    gb_p = nc.declare_dram_parameter("gb", [128, 4], F32, isOutput=False)
    out = nc.declare_dram_parameter("out", [2, 128, n], F32, isOutput=True)

    cs_dram = nc.dram_tensor("cs_scratch", [passes, ipp, 4, 512], F32)
    r_dram = nc.dram_tensor("r_scratch", [passes, ipp, 4, 512], F32)
    mr_dram = nc.dram_tensor("mr_scratch", [16, 2], F32)

    ADD = mybir.AluOpType.add
    MUL = mybir.AluOpType.mult
    SUB = mybir.AluOpType.subtract
    EXP = mybir.ActivationFunctionType.Exp
    SQRT = mybir.ActivationFunctionType.Sqrt
    COPY = mybir.ActivationFunctionType.Copy

    with tile.TileContext(nc) as tc:
        with tc.tile_pool(name="wpool", bufs=1) as wp, \
             tc.tile_pool(name="psum", space="PSUM", bufs=1) as pp, \
             tc.tile_pool(name="bigsb", bufs=1) as bp, \
             tc.tile_pool(name="epool", bufs=3) as ep, \
             tc.tile_pool(name="rpool", bufs=2) as rp, \
             tc.tile_pool(name="spool", bufs=1) as sp, \
             tc.tile_pool(name="opool", bufs=2) as op:

            def pvtile(name):
                return pp.tile([128, 512], F32, tag="pv", bufs=4, name=name,
                               uniquify=True)

            def zfill(t):
                pt, ft = t.shape[0], t.shape[-1]
                nc.scalar.activation(t, zeros_sb[:pt, :ft], COPY)

            wq_sb = wp.tile([128, 256], F32)
            wk_sb = wp.tile([128, 256], F32)
            wv_sb = wp.tile([128, 256], F32)
            wo_sb = wp.tile([128, 256], BF16)
            g16_sb = wp.tile([128, 32], F32)
            gb_sb = wp.tile([128, 4], F32)
            bq_sb = wp.tile([1, 128], F32)
            bk_sb = wp.tile([1, 128], F32)
            bv_sb = wp.tile([1, 128], F32)
            bo_sb = wp.tile([1, 256], F32)
            ones_n = wp.tile([1, 512], F32)
            zeros_sb = wp.tile([128, 512], F32)
            fence_sb = wp.tile([1, 1], F32)
            nc.vector.memset(ones_n, 1.0)
            nc.vector.memset(zeros_sb, 0.0)
            nc.vector.memset(fence_sb, 0.0)
            nc.sync.dma_start(out=wq_sb, in_=wq_p[:])
            nc.sync.dma_start(out=wk_sb, in_=wk_p[:])
            nc.sync.dma_start(out=wv_sb, in_=wv_p[:])
            nc.sync.dma_start(out=wo_sb, in_=wo_p[:])
            nc.sync.dma_start(out=g16_sb, in_=g16_p[:])
            nc.sync.dma_start(out=gb_sb, in_=gb_p[:])
            nc.sync.dma_start(out=bq_sb, in_=bq_p[:])
            nc.sync.dma_start(out=bk_sb, in_=bk_p[:])
            nc.sync.dma_start(out=bv_sb, in_=bv_p[:])
            nc.sync.dma_start(out=bo_sb, in_=bo_p[:])

            xq_sb = bp.tile([128, 2, n], F32)
            xkv_sb = bp.tile([128, 2, n], F32)
            q_sb = bp.tile([128, n], BF16)
            k_sb = bp.tile([128, n], BF16)
            vt_sb = bp.tile([128, n], BF16)
            attn_sb = bp.tile([128, n], BF16)
            y_sb = bp.tile([128, 2, n], F32)

            nc.sync.dma_start(out=xq_sb, in_=x_q[:])
            nc.sync.dma_start(out=xkv_sb, in_=x_kv[:])

            # PE warmups: absorb each input-DMA semaphore on its own matmul
            # (distinct columns of one PSUM tile -> no WAW between them).
            warm = pvtile("warm")
            warm_srcs = (xq_sb[:, 0, 0:1], xkv_sb[:, 0, 0:1],
                         wq_sb[:, 0:1], wk_sb[:, 0:1], wv_sb[:, 0:1],
                         g16_sb[:, 0:1], wo_sb[:, 0:1], bo_sb[0:1, 0:1],
                         zeros_sb[:, 0:1])
            for wi, wt in enumerate(warm_srcs):
                nc.tensor.matmul(warm[0:1, wi:wi + 1], wt, wt,
                                 start=True, stop=True, skip_group_check=True)

            # ---------- stage A: projections (vT first, so later q/k DVE
            # evacuation ticks cover the vT ticks for the main loop) ----------
            for j in range(jt):
                js = slice(128 * j, 128 * j + 128)
                pv = pvtile("pv")
                for cc in range(2):
                    nc.tensor.matmul(
                        pv[:, 0:128], xkv_sb[:, cc, js],
                        wv_sb[:, 128 * cc:128 * cc + 128],
                        start=(cc == 0), stop=False)
                nc.tensor.matmul(pv[:, 0:128], ones_n[0:1, 0:128], bv_sb,
                                 start=False, stop=True)
                nc.vector.tensor_copy(vt_sb[:, js], pv[:, 0:128])

            for nchunk in range(n // 512):
                s = slice(512 * nchunk, 512 * nchunk + 512)
                for (w_sb, b_sb, src, dst) in (
                    (wq_sb, bq_sb, xq_sb, q_sb),
                    (wk_sb, bk_sb, xkv_sb, k_sb),
                ):
                    ps = pvtile("ps")
                    for cc in range(2):
                        nc.tensor.matmul(
                            ps, w_sb[:, 128 * cc:128 * cc + 128],
                            src[:, cc, s], start=(cc == 0), stop=False)
                    nc.tensor.matmul(ps, b_sb, ones_n, start=False, stop=True)
                    nc.vector.tensor_copy(dst[:, s], ps)

            # DVE fence + absorber: the first (mode-switching) QK matmul must
            # carry a PE wait, so absorb the q/k-evacuation DVE tick here.
            nc.vector.tensor_copy(fence_sb, k_sb[0:1, n - 1:n])
            nc.tensor.matmul(warm[0:1, 9:10], fence_sb, fence_sb,
                             start=True, stop=True, skip_group_check=True)

            # ---------- main loop: QK -> exp -> PV ----------
            for p_i in range(passes):
                pvs = [pvtile(f"pvacc{p_i}_{i}") for i in range(ipp)]
                # ACT zero-fill: absorbs this PSUM slot's previous readers/
                # writers on the ACT queue; also provides the zero base the
                # start=False accumulation needs.
                for ic in range(ipp):
                    zfill(pvs[ic])
                for j in range(jt):
                    js = slice(128 * j, 128 * j + 128)
                    for ic in range(ipp):
                        i0 = 512 * (ipp * p_i + ic)
                        isl = slice(i0, i0 + 512)
                        sc = pp.tile([128, 2048], F32, tag="sc", bufs=1,
                                     name="sc")
                        for h in range(4):
                            hp = slice(32 * h, 32 * h + 16)
                            nc.tensor.matmul(
                                sc[:, 512 * h:512 * h + 512],
                                k_sb[hp, js], q_sb[hp, isl],
                                start=True, stop=True,
                                tile_position=(32 * h, 0))
                        e_t = ep.tile([128, 2048], BF16, tag="e", name="e_t")
                        nc.scalar.activation(e_t, sc, EXP)
                        for h in range(4):
                            nc.tensor.matmul(
                                pvs[ic][32 * h:32 * h + 32, :],
                                vt_sb[:, 128 * j + 32 * h:128 * j + 32 * h + 32],
                                e_t[:, 512 * h:512 * h + 512],
                                start=False, stop=(j == jt - 1),
                                tile_position=(0, 32 * h),
                                skip_group_check=True)
                # absorb the pending PE writes of each accumulator on a
                # single-wait matmul each, before any DVE reader touches them
                # (adds 0 to a padding-derived element; numerically inert).
                for ic in range(ipp):
                    nc.tensor.matmul(pvs[ic][0:1, 0:1], zeros_sb[0:1, 0:1],
                                     zeros_sb[0:1, 0:1], start=False, stop=False,
                                     skip_group_check=True)
                # pass epilogue: colsums -> reciprocal -> normalize
                for ic in range(ipp):
                    cs_sb = rp.tile([128, 512], F32, tag="cs", name="cs_sb")
                    nc.vector.tensor_copy(cs_sb, pvs[ic])
                    for h in range(4):
                        nc.sync.dma_start(
                            out=cs_dram[p_i, ic, h],
                            in_=cs_sb[32 * h + 16:32 * h + 17, :])
                csrows = ipp * 4 * 512 // 64
                cs_p = rp.tile([csrows, 64], F32, tag="csp", name="cs_p")
                nc.sync.dma_start(
                    out=cs_p,
                    in_=cs_dram[p_i].rearrange("a b (g f) -> (a b g) f", f=64))
                r_p = rp.tile([csrows, 64], F32, tag="csp", name="r_p")
                nc.vector.reciprocal(r_p, cs_p)
                nc.sync.dma_start(
                    out=r_dram[p_i].rearrange("a b (g f) -> (a b g) f", f=64),
                    in_=r_p)
                for ic in range(ipp):
                    i0 = 512 * (ipp * p_i + ic)
                    rr = rp.tile([128, 512], F32, tag="rr", name="rr")
                    nc.sync.dma_start(
                        out=rr,
                        in_=bass.AP(r_dram, (p_i * ipp + ic) * 4 * 512,
                                    [[512, 4], [0, 32], [1, 512]]))
                    nc.vector.tensor_tensor(
                        attn_sb[:, i0:i0 + 512], pvs[ic], rr, MUL)
                # DVE fence + absorber: a PE matmul whose only fresh
                # dependency is the latest DVE tick of this pass's epilogue
                # (RAW on the last attn slice orders the fence last).
                i0_last = 512 * (ipp * p_i + ipp - 1)
                nc.vector.tensor_copy(fence_sb,
                                      attn_sb[0:1, i0_last + 511:i0_last + 512])
                nc.tensor.matmul(pvs[0][0:1, 1:2], fence_sb, fence_sb,
                                 start=False, stop=False, skip_group_check=True)

            # ---------- stage C: out-proj + residual + groupnorm ----------
            for ic in range(ich):
                isl = slice(512 * ic, 512 * ic + 512)
                for ct in range(2):
                    pz = pvtile("pz")
                    nc.tensor.matmul(pz, wo_sb[:, 128 * ct:128 * ct + 128],
                                     attn_sb[:, isl], start=True, stop=False)
                    nc.tensor.matmul(pz, bo_sb[0:1, 128 * ct:128 * ct + 128],
                                     ones_n, start=False, stop=True)
                    nc.vector.tensor_tensor(
                        y_sb[:, ct, isl], pz, xq_sb[:, ct, isl], ADD)

            m1 = pvtile("m1")
            m2 = pvtile("m2")
            for ct in range(2):
                y2 = op.tile([128, n], F32, tag="y2", bufs=1, name="y2")
                nc.vector.tensor_tensor(y2, y_sb[:, ct, :], y_sb[:, ct, :], MUL)
                for ch in range(n // 512):
                    s = slice(512 * ch, 512 * ch + 512)
                    first = ct == 0 and ch == 0
                    last = ct == 1 and ch == n // 512 - 1
                    nc.tensor.matmul(m1[:16, :], g16_sb[:, 16 * ct:16 * ct + 16],
                                     y_sb[:, ct, s], start=first, stop=last)
                    nc.tensor.matmul(m2[:16, :], g16_sb[:, 16 * ct:16 * ct + 16],
                                     y2[:, s], start=first, stop=last)

            mv = sp.tile([16, 2], F32, name="mv")
            nc.vector.reduce_sum(mv[:, 0:1], m1[:16, :],
                                 axis=mybir.AxisListType.X)
            nc.vector.reduce_sum(mv[:, 1:2], m2[:16, :],
                                 axis=mybir.AxisListType.X)
            mean = sp.tile([16, 1], F32, name="mean")
            e2 = sp.tile([16, 1], F32, name="e2")
            var = sp.tile([16, 1], F32, name="var")
            sd = sp.tile([16, 1], F32, name="sd")
            rstd = sp.tile([16, 1], F32, name="rstd")
            eps_t = sp.tile([16, 1], F32, name="eps_t")
            mr = sp.tile([16, 2], F32, name="mr")
            nc.vector.memset(eps_t, EPS)
            nc.vector.tensor_scalar_mul(mean, mv[:, 0:1], 1.0 / gn_cnt)
            nc.vector.tensor_scalar_mul(e2, mv[:, 1:2], 1.0 / gn_cnt)
            nc.vector.tensor_tensor(var, mean, mean, MUL)
            nc.vector.tensor_tensor(var, e2, var, SUB)
            nc.scalar.activation(sd, var, SQRT, bias=eps_t)
            nc.vector.reciprocal(rstd, sd)
            nc.vector.tensor_copy(mr[:, 0:1], mean)
            nc.vector.tensor_copy(mr[:, 1:2], rstd)
            nc.sync.dma_start(out=mr_dram[:], in_=mr)

            for ct in range(2):
                mrb = sp.tile([128, 2], F32, tag="mrb", name="mrb")
                nc.sync.dma_start(
                    out=mrb,
                    in_=bass.AP(mr_dram, 16 * ct, [[2, 8], [0, 16], [1, 2]]))
                rg = sp.tile([128, 1], F32, tag="rg", name="rg")
                bb = sp.tile([128, 1], F32, tag="bb", name="bb")
                nc.vector.tensor_tensor(rg, mrb[:, 1:2],
                                        gb_sb[:, 2 * ct:2 * ct + 1], MUL)
                nc.vector.tensor_tensor(bb, mrb[:, 0:1], rg, MUL)
                nc.vector.tensor_tensor(bb, gb_sb[:, 2 * ct + 1:2 * ct + 2],
                                        bb, SUB)
                for half in range(max(1, n // 2048)):
                    hs = slice(2048 * half, min(2048 * half + 2048, n))
                    o_t = op.tile([128, 2048], F32, tag="o", name="o_t")
                    width = hs.stop - hs.start
                    nc.vector.tensor_scalar(
                        o_t[:, :width], y_sb[:, ct, hs], rg, bb, MUL, ADD)
                    nc.sync.dma_start(out=out[ct][:, hs], in_=o_t[:, :width])
    nc.finalize()
    return nc


# ---------------- host side ----------------

def _prep_core(x_q, x_kv, wq, bq, wk, bk, wv, bv, wo, bo, gamma, beta):
    d = {}
    d["x_q"] = np.ascontiguousarray(
        x_q.reshape(2, 128, -1).transpose(1, 0, 2)).astype(np.float32)
    d["x_kv"] = np.ascontiguousarray(
        x_kv.reshape(2, 128, -1).transpose(1, 0, 2)).astype(np.float32)

    def lhsT_packed(w, scale):
        lt = np.zeros((C, 128), np.float32)
        for h in range(HEADS):
            lt[:, 32 * h:32 * h + HD] = scale * w[HD * h:HD * h + HD, :].T
        return np.ascontiguousarray(
            lt.reshape(2, 128, 128).transpose(1, 0, 2).reshape(128, 256))

    d["wq"] = lhsT_packed(wq, SCALE)
    d["wk"] = lhsT_packed(wk, 1.0)

    def brow(b, scale):
        r = np.zeros((1, 128), np.float32)
        for h in range(HEADS):
            r[0, 32 * h:32 * h + HD] = scale * b[HD * h:HD * h + HD]
        return r

    d["bq"] = brow(bq, SCALE)
    d["bk"] = brow(bk, 1.0)

    wv_aug = np.zeros((C, 128), np.float32)
    bv_aug = np.zeros((1, 128), np.float32)
    for h in range(HEADS):
        wv_aug[:, 32 * h:32 * h + HD] = wv[HD * h:HD * h + HD, :].T
        bv_aug[0, 32 * h:32 * h + HD] = bv[HD * h:HD * h + HD]
        bv_aug[0, 32 * h + HD] = 1.0
    d["wv"] = np.ascontiguousarray(
        wv_aug.reshape(2, 128, 128).transpose(1, 0, 2).reshape(128, 256))
    d["bv"] = bv_aug

    wo_pad = np.zeros((128, C), np.float32)  # [r=32h+d, c]
    for h in range(HEADS):
        wo_pad[32 * h:32 * h + HD, :] = wo[:, HD * h:HD * h + HD].T
    d["wo"] = np.ascontiguousarray(wo_pad).astype(ml_dtypes.bfloat16)
    d["bo"] = bo.reshape(1, 256).astype(np.float32)

    g16 = np.zeros((128, 32), np.float32)
    for ct in range(2):
        for r in range(128):
            g16[r, 16 * ct + 8 * ct + r // 16] = 1.0
    d["g16"] = g16
    gb = np.zeros((128, 4), np.float32)
    for ct in range(2):
        gb[:, 2 * ct] = gamma.reshape(2, 128)[ct]
        gb[:, 2 * ct + 1] = beta.reshape(2, 128)[ct]
    d["gb"] = gb
    return d


_CACHE = {}


def _get_nc(n=N):
    if n not in _CACHE:
        _CACHE[n] = build_nc(n)
    return _CACHE[n]


class _Runner:
    """run_bass_via_pjrt with the jitted executable cached across calls."""

    def __init__(self, nc, n_cores=NCORES):
        import jax
        import jax.numpy as jnp
        from jax.sharding import Mesh, PartitionSpec
        from jax.experimental.shard_map import shard_map
        from concourse import bass2jax
        from concourse import mybir as mb

        bass2jax.install_neuronx_cc_hook()
        self.nc = nc
        self.n_cores = n_cores
        partition_name = (nc.partition_id_tensor.name
                          if nc.partition_id_tensor else None)
        in_names, out_names, out_avals, zero_outs = [], [], [], []
        for alloc in nc.m.functions[0].allocations:
            if not isinstance(alloc, mb.MemoryLocationSet):
                continue
            name = alloc.memorylocations[0].name
            if alloc.kind == "ExternalInput":
                if name != partition_name:
                    in_names.append(name)
            elif alloc.kind == "ExternalOutput":
                out_names.append(name)
                shape = tuple(alloc.tensor_shape)
                dtype = mb.dt.np(alloc.dtype)
                out_avals.append(jax.core.ShapedArray(shape, dtype))
                zero_outs.append(np.zeros(shape, dtype))
        self.in_names, self.out_names = in_names, out_names
        self.zero_outs = zero_outs
        n_params, n_outs = len(in_names), len(out_names)
        donate = tuple(range(n_params, n_params + n_outs))

        def _body(*args):
            operands = list(args)
            all_in_names = list(in_names) + list(out_names)
            if partition_name is not None:
                operands.append(bass2jax.partition_id_tensor())
                all_in_names.append(partition_name)
            outs = bass2jax._bass_exec_p.bind(
                *operands,
                out_avals=tuple(out_avals),
                in_names=tuple(all_in_names),
                out_names=tuple(out_names),
                lowering_input_output_aliases=(),
                sim_require_finite=True,
                sim_require_nnan=True,
                nc=nc,
            )
            return tuple(outs)

        devices = jax.devices()[:n_cores]
        mesh = Mesh(np.asarray(devices), ("core",))
        in_specs = (PartitionSpec("core"),) * (n_params + n_outs)
        out_specs = (PartitionSpec("core"),) * n_outs
        self.fn = jax.jit(
            shard_map(_body, mesh=mesh, in_specs=in_specs,
                      out_specs=out_specs, check_rep=False),
            donate_argnums=donate, keep_unused=True)

    def bench(self, in_maps, iters=8):
        """Per-iteration device time: inputs resident on device, async
        dispatch of `iters` executions, single block at the end."""
        import jax, time
        ins = [
            jax.device_put(
                np.concatenate([np.asarray(m[name]) for m in in_maps], axis=0))
            for name in self.in_names
        ]
        for x in ins:
            x.block_until_ready()
        zout_sets = []
        for _ in range(iters + 1):
            zouts = [jax.device_put(np.concatenate([z] * self.n_cores, axis=0))
                     for z in self.zero_outs]
            for z in zouts:
                z.block_until_ready()
            zout_sets.append(zouts)
        # warmup
        outs = self.fn(*ins, *zout_sets[0])
        for o in outs:
            o.block_until_ready()
        t0 = time.perf_counter()
        all_outs = []
        for i in range(iters):
            all_outs.append(self.fn(*ins, *zout_sets[1 + i]))
        for o in all_outs[-1]:
            o.block_until_ready()
        dt = (time.perf_counter() - t0) / iters
        return dt

    def __call__(self, in_maps, block=True):
        ins = [
            np.concatenate([np.asarray(m[name]) for m in in_maps], axis=0)
            for name in self.in_names
        ]
        zouts = [np.concatenate([z] * self.n_cores, axis=0)
                 for z in self.zero_outs]
        outs = self.fn(*ins, *zouts)
        if block:
            for o in outs:
                o.block_until_ready()
        per_core = []
        for c in range(self.n_cores):
            d = {}
            for name, arr, zo in zip(self.out_names, outs, self.zero_outs):
                k = zo.shape[0]
                d[name] = np.asarray(arr[c * k:(c + 1) * k])
            per_core.append(d)
        return per_core


_RUNNER = {}


def get_runner(n=N):
    if n not in _RUNNER:
        _RUNNER[n] = _Runner(_get_nc(n))
    return _RUNNER[n]


def run_cores(in_maps, n=N):
    return get_runner(n)(in_maps)


def make_in_maps(feat_a, feat_b, weights):
    w = weights
    in_maps = []
    for core in range(NCORES):
        br, b = core // 4, core % 4
        if br == 0:
            d = _prep_core(
                feat_a[b].reshape(C, -1), feat_b[b].reshape(C, -1),
                w["q_a_w"], w["q_a_b"], w["k_b_w"], w["k_b_b"],
                w["v_b_w"], w["v_b_b"], w["out_a_w"], w["out_a_b"],
                w["norm_a_g"], w["norm_a_b"])
        else:
            d = _prep_core(
                feat_b[b].reshape(C, -1), feat_a[b].reshape(C, -1),
                w["q_b_w"], w["q_b_b"], w["k_a_w"], w["k_a_b"],
                w["v_a_w"], w["v_a_b"], w["out_b_w"], w["out_b_b"],
                w["norm_b_g"], w["norm_b_b"])
        in_maps.append({k: np.ascontiguousarray(v) for k, v in d.items()})
    return in_maps


def kernel(**inputs):
    feat_a = np.asarray(inputs["feat_a"], np.float32)
    feat_b = np.asarray(inputs["feat_b"], np.float32)
    in_maps = make_in_maps(feat_a, feat_b, inputs)
    results = run_cores(in_maps)

    def unpack(r):
        return r["out"].reshape(C, HW, HW)

    a_out = np.stack([unpack(results[b]) for b in range(4)])
    b_out = np.stack([unpack(results[4 + b]) for b in range(4)])
    return (a_out, b_out)
